# revision 26
# baseline (speedup 1.0000x reference)
"""Multi-head causal attention (B=2, S=2048, E=1024, H=16) on 8 TRN2 cores.

Sharding: 2-way data parallel on batch x 4-way tensor parallel on heads.
Core c handles batch b = c//4 and heads [4g, 4g+4) where g = c%4.
Each core computes q/k/v projections for its 4 heads, causal attention,
and a partial output projection (row-parallel Wo slice); the host sums
the 4 partials per batch and adds bo.

v2 design notes (vs the fp32r v1):
- All matmul operands are bf16 (psum accumulation stays fp32): halves
  input DMA bytes, enables FWL weight loads, and avoids the fp32r
  narrow-N penalty.  The 1/sqrt(D) score scale is folded into the exp
  activation's free affine (scale=0.125) instead of pre-scaling Wq.
- Scores are computed transposed (k on partitions, q on free dim) so the
  softmax denominator comes free as an extra ones-column in the P@V
  lhsT.  The two heads of a pair run as concurrent row-tiled matmuls
  (K=64 at array rows 0:64 / 64:128).
- Causal trimming: for diagonal chunks (d0>=0) only columns [d0, 512)
  are computed, and the partially-masked region is always exactly the
  first 128 columns -> one constant [128,128] lower-triangular mask.
- Score chunks are processed in merge-groups of 2: both chunks' scores
  land bank-aligned in one [128,2048] psum tile and ONE exp activation
  covers the whole group (scalar-engine call count ~40 instead of ~100).
- Softmax-normalize reciprocals read the PV psum directly; the
  denominator broadcast DMAs ride the Activation HWDGE queue so they
  never queue behind output DMAs on the SP queue.
- Emission order = tile-scheduler priority: each attention section is
  preceded only by the projection units it needs; later projections,
  v-chunks and out-projection units are emitted after it and fill PE
  stalls (keeps HAM warm).
"""

import sys

sys.path.insert(0, "/opt/trn_rl_repo")

from contextlib import ExitStack

import ml_dtypes
import numpy as np

import concourse.bass as bass  # noqa: F401  (registers engines)
from concourse.ap import AP as _AP


def _free_bcast(src_ap, n):
    """View a [1, F] AP as [1, n, F] with a zero-stride middle dim (DMA replicate)."""
    return _AP(
        src_ap.tensor, src_ap.offset,
        [list(p) for p in src_ap.ap[:1]] + [[0, n]] + [list(p) for p in src_ap.ap[1:]],
    )

import concourse.tile as tile
from concourse import bacc, mybir
from concourse.bass_utils import run_bass_kernel_spmd

B, S, E, H = 2, 2048, 1024, 16
D = E // H            # 64
HPC = H // 4          # 4 heads per core
EC = HPC * D          # 256 = per-core head-dim width
NQT = S // 512        # 4 q-tiles of 512
NKC = S // 128        # 16 k-chunks of 128
NEC = E // 128        # 8 E-chunks of 128

F32 = mybir.dt.float32
BF16 = mybir.dt.bfloat16
EXP = mybir.ActivationFunctionType.Exp

# constants blob [128, 193] bf16:
#   cols 0:128   lower-triangular mask  M[kk, jj] = 1.0 if jj >= kk else 0
#   cols 128:193 static v_sb block [1, 1, 0*63]
CST_W = 193

# v_sb per k-chunk: [128, 386] bf16
#   h0: cols 0:64 v, 64 ones                   -> lhsT [0:65]   M=65  (den row 64)
#   h1: col 65 ones, 66:129 zeros, 129:193 v   -> lhsT [65:193] M=128 (den row 0)
#   h2: cols 193:257 v, 257 ones               -> lhsT [193:258]
#   h3: col 258 ones, 259:322 zeros, 322:386 v -> lhsT [258:386]
V_W = 386
V_DATA = [0, 129, 193, 322]
V_LHS = [(0, 65), (65, 193), (193, 258), (258, 386)]
V_STATIC = [64, 257]


def _build_nc():
    nc = bacc.Bacc("TRN2", target_bir_lowering=False, debug=False, num_devices=8)

    xT = nc.dram_tensor("xT", [E, S], BF16, kind="ExternalInput")
    wq = nc.dram_tensor("wq", [E, EC], BF16, kind="ExternalInput")
    wk = nc.dram_tensor("wk", [E, EC], BF16, kind="ExternalInput")
    wv = nc.dram_tensor("wv", [E, EC], BF16, kind="ExternalInput")
    wo = nc.dram_tensor("wo", [EC, E], BF16, kind="ExternalInput")
    bqd = nc.dram_tensor("bq", [EC], F32, kind="ExternalInput")
    bkd = nc.dram_tensor("bk", [EC], F32, kind="ExternalInput")
    bvd = nc.dram_tensor("bv", [EC], BF16, kind="ExternalInput")
    cst = nc.dram_tensor("cst", [128, CST_W], BF16, kind="ExternalInput")
    out = nc.dram_tensor("out", [S, E], BF16, kind="ExternalOutput")

    with tile.TileContext(nc) as tc:
        with ExitStack() as stack:
            cpool = stack.enter_context(tc.tile_pool(name="const", bufs=1))
            wpool = stack.enter_context(tc.tile_pool(name="w", bufs=1))
            xpool = stack.enter_context(tc.tile_pool(name="xt", bufs=NEC))
            qkpool = stack.enter_context(tc.tile_pool(name="qkt", bufs=4))
            vpool = stack.enter_context(tc.tile_pool(name="vsb", bufs=NKC))
            apool = stack.enter_context(tc.tile_pool(name="asb", bufs=2))
            ppool = stack.enter_context(tc.tile_pool(name="psb", bufs=18))
            rspool = stack.enter_context(tc.tile_pool(name="rs", bufs=2))
            bcpool = stack.enter_context(tc.tile_pool(name="bc", bufs=2))
            opool = stack.enter_context(tc.tile_pool(name="osb", bufs=4))
            pj_ps = stack.enter_context(tc.tile_pool(name="pj_ps", bufs=2, space="PSUM"))
            qk_ps = stack.enter_context(tc.tile_pool(name="qk_ps", bufs=2, space="PSUM"))
            at_ps = stack.enter_context(tc.tile_pool(name="at_ps", bufs=2, space="PSUM"))

            # ---- weight + input DMAs ----
            # few large queue entries (the DGE burns ~600ns per entry), ordered
            # so the first attention section's inputs land first: wq, wk, then
            # the first column-half of xT, then constants/wv/wo, then the rest.
            w_sb = {}
            w_big = {}
            for name, dram in (("q", wq), ("k", wk), ("v", wv)):
                big = wpool.tile([128, NEC * EC], BF16, tag=f"w{name}")
                w_big[name] = (big, dram)
                w_sb[name] = [big[:, e * EC:(e + 1) * EC] for e in range(NEC)]

            def w_dma(name):
                big, dram = w_big[name]
                nc.sync.dma_start(
                    big[:].rearrange("p (c n) -> p c n", c=NEC),
                    dram.ap().rearrange("(c p) n -> p c n", p=128),
                )

            bq_sb = cpool.tile([128, 2], F32, tag="bq")
            nc.sync.dma_start(bq_sb[:], bqd.ap().rearrange("(b p) -> p b", p=128))
            # preload the exp table set immediately (it costs ~2.7us once)
            dummy0 = cpool.tile([1, 1], F32, tag="dummy0")
            nc.scalar.activation(dummy0[:], bq_sb[0:1, 0:1], EXP)
            # HAM warmup: ~64 tiny matmuls keep the PE "busy" through the
            # DMA-bound startup so the clock gate opens before real matmuls
            warm_ps = pj_ps.tile([128, 512], F32, tag="pj")
            for _ in range(64):
                nc.tensor.matmul(
                    warm_ps[0:2, 0:2], bq_sb[:, 0:2], bq_sb[:, 0:2],
                    start=True, stop=True,
                )

            w_dma("q")
            w_dma("k")
            xt_sb = [
                xpool.tile([128, S], BF16, tag="xt", name=f"xt{e}")
                for e in range(NEC)
            ]
            for e in range(NEC):
                nc.sync.dma_start(xt_sb[e][:, 0:1024], xT[e * 128:(e + 1) * 128, 0:1024])

            cst_sb = cpool.tile([128, CST_W], BF16, tag="cst")
            nc.sync.dma_start(cst_sb[:], cst[:])
            mask_sb = cst_sb[:, 0:128]
            static_blk = cst_sb[:, 128:193]
            bk_sb = cpool.tile([128, 2], F32, tag="bk")
            nc.sync.dma_start(bk_sb[:], bkd.ap().rearrange("(b p) -> p b", p=128))
            bv_sb = cpool.tile([1, EC], BF16, tag="bv")
            nc.sync.dma_start(bv_sb[:], bvd.ap().rearrange("(o n) -> o n", o=1))

            w_dma("v")
            wo_big = cpool.tile([128, 2 * E], BF16, tag="wo")
            nc.sync.dma_start(
                wo_big[:].rearrange("p (c n) -> p c n", c=2),
                wo.ap().rearrange("(c p) n -> p c n", p=128),
            )
            wo_sb = [wo_big[:, j * E:(j + 1) * E] for j in range(2)]
            for e in range(NEC):
                nc.sync.dma_start(
                    xt_sb[e][:, 1024:2048], xT[e * 128:(e + 1) * 128, 1024:2048]
                )

            # bv broadcast [128, EC] = ones[1,128].T @ bv[1,EC]
            # (mask row 0 is all-ones and doubles as the ones lhsT row)
            bvb_ps = pj_ps.tile([128, 512], F32, tag="pj")
            nc.tensor.matmul(
                bvb_ps[:, 0:EC], cst_sb[0:1, 0:128], bv_sb[:], start=True, stop=True
            )
            bvb_sb = cpool.tile([128, EC], F32, tag="bvb")
            nc.vector.tensor_copy(bvb_sb[:], bvb_ps[:, 0:EC])

            qt_sb = [qkpool.tile([128, S], BF16, tag="qkt", name=f"qt{i}") for i in range(2)]
            kt_sb = [qkpool.tile([128, S], BF16, tag="qkt", name=f"kt{i}") for i in range(2)]
            a_sb = [apool.tile([128, S], BF16, tag="asb", name=f"a{i}") for i in range(2)]

            def qk_unit(name, dst, pb, ti, bias_sb):
                """One q/k projection unit: dst[pb][:, ti*512:+512] (d on partitions)."""
                ps = pj_ps.tile([128, 512], F32, tag="pj")
                for e in range(NEC):
                    nc.tensor.matmul(
                        ps[:],
                        w_sb[name][e][:, pb * 128:(pb + 1) * 128],
                        xt_sb[e][:, ti * 512:(ti + 1) * 512],
                        start=(e == 0),
                        stop=(e == NEC - 1),
                    )
                nc.vector.tensor_scalar_add(
                    dst[pb][:, ti * 512:(ti + 1) * 512], ps[:], bias_sb[:, pb:pb + 1]
                )

            v_sb = []

            def v_unit(m):
                """v projection for k-chunk m (k on partitions, PV-ready layout)."""
                vt = vpool.tile([128, V_W], BF16, tag="vsb")
                for cs in V_STATIC:
                    nc.vector.tensor_copy(vt[:, cs:cs + 65], static_blk)
                vps = pj_ps.tile([128, 512], F32, tag="pj")
                for e in range(NEC):
                    nc.tensor.matmul(
                        vps[:, 0:EC],
                        xt_sb[e][:, m * 128:(m + 1) * 128],
                        w_sb["v"][e][:],
                        start=(e == 0),
                        stop=(e == NEC - 1),
                    )
                for h in range(HPC):
                    d0 = V_DATA[h]
                    nc.vector.tensor_add(
                        vt[:, d0:d0 + 64],
                        vps[:, h * 64:(h + 1) * 64],
                        bvb_sb[:, h * 64:(h + 1) * 64],
                    )
                v_sb.append(vt)

            def oproj_unit(m):
                """out[m*128:+128, :] = sum_j a_sb[j].T @ wo_sb[j], staged into
                one [128, 1024] row-contiguous DMA (2KB rows, full bandwidth)."""
                osb = opool.tile([128, E], BF16, tag="osb")
                for nh in range(2):
                    ops = pj_ps.tile([128, 512], F32, tag="pj")
                    for j in range(2):
                        nc.tensor.matmul(
                            ops[:],
                            a_sb[j][:, m * 128:(m + 1) * 128],
                            wo_sb[j][:, nh * 512:(nh + 1) * 512],
                            start=(j == 0),
                            stop=(j == 1),
                        )
                    nc.vector.tensor_copy(osb[:, nh * 512:(nh + 1) * 512], ops[:])
                eng = nc.sync if m % 2 == 0 else nc.scalar
                eng.dma_start(out[m * 128:(m + 1) * 128, :], osb[:])

            def attn_section(p, ti, mid=None):
                nchunks = 4 * (ti + 1)
                lhs = (V_LHS[2 * p], V_LHS[2 * p + 1])
                ape = at_ps.tile([128, 512], F32, tag="at", name="ape")
                apo = at_ps.tile([128, 512], F32, tag="at", name="apo")
                tgt = (ape[0:65, :], apo[:, :])

                # phase 1: all scores + exp + mask, so the exp stream never
                # queues behind PV matmuls that wait on the previous section's
                # normalize chain (ape/apo pool release)
                psbs = []
                for ci in range(nchunks):
                    d0 = ci * 128 - ti * 512
                    q0 = max(d0, 0)
                    qkp = qk_ps.tile([128, 1024], F32, tag="qk")
                    psb = ppool.tile([128, 1024], BF16, tag="psb")
                    psbs.append((psb, q0))
                    for hh in range(2):
                        nc.tensor.matmul(
                            qkp[:, 512 * hh + q0:512 * hh + 512],
                            kt_sb[p][64 * hh:64 * hh + 64, ci * 128:(ci + 1) * 128],
                            qt_sb[p][64 * hh:64 * hh + 64, ti * 512 + q0:(ti + 1) * 512],
                            start=True, stop=True,
                        )
                    if q0 == 0:
                        nc.scalar.activation(psb[:], qkp[:], EXP, scale=0.125)
                    else:
                        nc.scalar.activation(
                            psb[:, q0:512], qkp[:, q0:512], EXP, scale=0.125)
                        nc.scalar.activation(
                            psb[:, 512 + q0:1024], qkp[:, 512 + q0:1024], EXP,
                            scale=0.125)
                    if d0 >= 0:
                        for hh in range(2):
                            nc.vector.tensor_mul(
                                psb[:, 512 * hh + q0:512 * hh + q0 + 128],
                                psb[:, 512 * hh + q0:512 * hh + q0 + 128],
                                mask_sb,
                            )
                # filler emission point: lower priority than the exp stream
                # above, higher than the PV drains below
                if mid is not None:
                    mid()
                # phase 2: PV accumulation trails the exp stream
                for ci in range(nchunks):
                    psb, q0 = psbs[ci]
                    for hh in range(2):
                        lh = lhs[hh]
                        nc.tensor.matmul(
                            tgt[hh][:, q0:512],
                            v_sb[ci][:, lh[0]:lh[1]],
                            psb[:, 512 * hh + q0:512 * hh + 512],
                            start=(ci == 0),
                            stop=(ci == nchunks - 1),
                        )

                # softmax normalization: stage denominators to SBUF, fast
                # recip, DMA broadcast, then per-head column scaling
                # (reciprocal_approx_fast misbehaves on a PSUM source on HW)
                ssb = rspool.tile([128, 512], F32, tag="ssb", name="ssb")
                rsf = rspool.tile([128, 512], F32, tag="rsf", name="rsf")
                nc.vector.tensor_copy(ssb[64:65, :], ape[64:65, :])
                nc.vector.tensor_copy(ssb[0:1, :], apo[0:1, :])
                nc.vector.reciprocal_approx_fast(out=rsf[0:65, :], in_=ssb[0:65, :])
                bcs = bcpool.tile([128, 512], F32, tag="bc", name="bcs")
                nc.scalar.dma_start(bcs[0:64, :], _free_bcast(rsf[64:65, :], 64))
                nc.scalar.dma_start(bcs[64:128, :], _free_bcast(rsf[0:1, :], 64))
                tcols = slice(ti * 512, (ti + 1) * 512)
                nc.vector.tensor_mul(a_sb[p][0:64, tcols], ape[0:64, :], bcs[0:64, :])
                nc.vector.tensor_mul(a_sb[p][64:128, tcols], apo[64:128, :], bcs[64:128, :])

            # ---- emission order = scheduler priority ----
            # each section's phase-1 (scores+exp) is preceded only by the q/k
            # units it needs; v-projections and out-projections are emitted at
            # the section's mid-point so they fill PE slack during the exp
            # stream without ever delaying it.
            def mk_mid(vs, qks, ops):
                def mid():
                    for name, pb, ti in qks:
                        qk_unit(name, qt_sb if name == "q" else kt_sb, pb, ti,
                                bq_sb if name == "q" else bk_sb)
                    for m in vs:
                        v_unit(m)
                    for m in ops:
                        oproj_unit(m)
                return mid

            qk_unit("q", qt_sb, 0, 0, bq_sb)
            qk_unit("k", kt_sb, 0, 0, bk_sb)
            attn_section(0, 0, mk_mid([0, 1, 2, 3], [("q", 1, 0), ("k", 1, 0)], []))
            attn_section(1, 0, mk_mid([], [("q", 0, 1), ("k", 0, 1)], []))
            attn_section(0, 1, mk_mid([4, 5, 6, 7], [("q", 1, 1), ("k", 1, 1)], []))
            attn_section(1, 1, mk_mid([], [("q", 0, 2), ("k", 0, 2)], [0, 1, 2, 3]))
            attn_section(0, 2, mk_mid([8, 9, 10, 11], [("q", 1, 2), ("k", 1, 2)], []))
            attn_section(1, 2, mk_mid([], [("q", 0, 3), ("k", 0, 3)], [4, 5, 6, 7]))
            attn_section(0, 3, mk_mid([12, 13, 14, 15], [("q", 1, 3), ("k", 1, 3)], []))
            attn_section(1, 3, mk_mid([], [], [8, 9, 10, 11]))
            for m in range(12, 16):
                oproj_unit(m)

    nc.compile()
    return nc


_NC = None


def _get_nc():
    global _NC
    if _NC is None:
        _NC = _build_nc()
    return _NC


def _constants():
    kk = np.arange(128)[:, None]
    jj = np.arange(128)[None, :]
    cst = np.zeros((128, CST_W), dtype=np.float32)
    cst[:, 0:128] = (jj >= kk).astype(np.float32)
    cst[:, 128] = 1.0
    cst[:, 129] = 1.0
    return cst.astype(ml_dtypes.bfloat16)


def _in_maps(inputs, Wq, bq, Wk, bk, Wv, bv, Wo, bo):
    bf = ml_dtypes.bfloat16
    inputs = np.asarray(inputs, dtype=np.float32)
    Wq = np.asarray(Wq, dtype=np.float32)
    Wk = np.asarray(Wk, dtype=np.float32)
    Wv = np.asarray(Wv, dtype=np.float32)
    Wo = np.asarray(Wo, dtype=np.float32)
    bq = np.asarray(bq, dtype=np.float32)
    bk = np.asarray(bk, dtype=np.float32)
    bv = np.asarray(bv, dtype=np.float32)

    cst = _constants()
    xTb = [np.ascontiguousarray(inputs[b].T).astype(bf) for b in range(B)]
    maps = []
    for c in range(8):
        b, g = divmod(c, 4)
        sl = slice(g * EC, (g + 1) * EC)
        maps.append({
            "xT": xTb[b],
            "wq": np.ascontiguousarray(Wq[:, sl]).astype(bf),
            "bq": np.ascontiguousarray(bq[sl]),
            "wk": np.ascontiguousarray(Wk[:, sl]).astype(bf),
            "bk": np.ascontiguousarray(bk[sl]),
            "wv": np.ascontiguousarray(Wv[:, sl]).astype(bf),
            "bv": np.ascontiguousarray(bv[sl]).astype(bf),
            "wo": np.ascontiguousarray(Wo[sl, :]).astype(bf),
            "cst": cst,
        })
    return maps


def _assemble(results, bo):
    bo = np.asarray(bo, dtype=np.float32)
    outs = [np.asarray(r["out"]).astype(np.float32) for r in results]
    full = np.empty((B, S, E), dtype=np.float32)
    for b in range(B):
        full[b] = outs[4 * b] + outs[4 * b + 1] + outs[4 * b + 2] + outs[4 * b + 3]
        full[b] += bo
    return full


def kernel(inputs, Wq, bq, Wk, bk, Wv, bv, Wo, bo):
    nc = _get_nc()
    maps = _in_maps(inputs, Wq, bq, Wk, bk, Wv, bv, Wo, bo)
    res = run_bass_kernel_spmd(nc, maps, list(range(8)))
    return _assemble(res.results, bo)


# revision 36
# speedup vs baseline: 1.4073x; 1.4073x over previous
"""Multi-head causal attention (B=2, S=2048, E=1024, H=16) on 8 TRN2 cores.

Sharding: 2-way data parallel on batch x 4-way tensor parallel on heads.
Core c handles batch b = c//4 and heads [4g, 4g+4) where g = c%4.
Each core computes q/k/v projections for its 4 heads, causal attention,
and a partial output projection (row-parallel Wo slice); the host sums
the 4 partials per batch and adds bo.

v2 design notes (vs the fp32r v1):
- All matmul operands are bf16 (psum accumulation stays fp32): halves
  input DMA bytes, enables FWL weight loads, and avoids the fp32r
  narrow-N penalty.  The 1/sqrt(D) score scale is folded into the exp
  activation's free affine (scale=0.125) instead of pre-scaling Wq.
- Scores are computed transposed (k on partitions, q on free dim) so the
  softmax denominator comes free as an extra ones-column in the P@V
  lhsT.  The two heads of a pair run as concurrent row-tiled matmuls
  (K=64 at array rows 0:64 / 64:128).
- Causal trimming: for diagonal chunks (d0>=0) only columns [d0, 512)
  are computed, and the partially-masked region is always exactly the
  first 128 columns -> one constant [128,128] lower-triangular mask.
- Score chunks are processed in merge-groups of 2: both chunks' scores
  land bank-aligned in one [128,2048] psum tile and ONE exp activation
  covers the whole group (scalar-engine call count ~40 instead of ~100).
- Softmax-normalize reciprocals read the PV psum directly; the
  denominator broadcast DMAs ride the Activation HWDGE queue so they
  never queue behind output DMAs on the SP queue.
- Emission order = tile-scheduler priority: each attention section is
  preceded only by the projection units it needs; later projections,
  v-chunks and out-projection units are emitted after it and fill PE
  stalls (keeps HAM warm).
"""

import sys

sys.path.insert(0, "/opt/trn_rl_repo")

from contextlib import ExitStack

import ml_dtypes
import numpy as np

import concourse.bass as bass  # noqa: F401  (registers engines)
from concourse.ap import AP as _AP


def _free_bcast(src_ap, n):
    """View a [1, F] AP as [1, n, F] with a zero-stride middle dim (DMA replicate)."""
    return _AP(
        src_ap.tensor, src_ap.offset,
        [list(p) for p in src_ap.ap[:1]] + [[0, n]] + [list(p) for p in src_ap.ap[1:]],
    )

import concourse.tile as tile
from concourse import bacc, mybir
from concourse.bass_utils import run_bass_kernel_spmd

B, S, E, H = 2, 2048, 1024, 16
D = E // H            # 64
HPC = H // 4          # 4 heads per core
EC = HPC * D          # 256 = per-core head-dim width
NQT = S // 512        # 4 q-tiles of 512
NKC = S // 128        # 16 k-chunks of 128
NEC = E // 128        # 8 E-chunks of 128

F32 = mybir.dt.float32
BF16 = mybir.dt.bfloat16
EXP = mybir.ActivationFunctionType.Exp

# constants blob [128, 193] bf16:
#   cols 0:128   lower-triangular mask  M[kk, jj] = 1.0 if jj >= kk else 0
#   cols 128:193 static v_sb block [1, 1, 0*63]
CST_W = 193

# v_sb per k-chunk: [128, 386] bf16
#   h0: cols 0:64 v, 64 ones                   -> lhsT [0:65]   M=65  (den row 64)
#   h1: col 65 ones, 66:129 zeros, 129:193 v   -> lhsT [65:193] M=128 (den row 0)
#   h2: cols 193:257 v, 257 ones               -> lhsT [193:258]
#   h3: col 258 ones, 259:322 zeros, 322:386 v -> lhsT [258:386]
V_W = 386
V_DATA = [0, 129, 193, 322]
V_LHS = [(0, 65), (65, 193), (193, 258), (258, 386)]
V_STATIC = [64, 257]


def _build_nc():
    nc = bacc.Bacc("TRN2", target_bir_lowering=False, debug=False, num_devices=8)

    xT = nc.dram_tensor("xT", [E, S], BF16, kind="ExternalInput")
    wq = nc.dram_tensor("wq", [E, EC], BF16, kind="ExternalInput")
    wk = nc.dram_tensor("wk", [E, EC], BF16, kind="ExternalInput")
    wv = nc.dram_tensor("wv", [E, EC], BF16, kind="ExternalInput")
    wo = nc.dram_tensor("wo", [EC, E], BF16, kind="ExternalInput")
    bqd = nc.dram_tensor("bq", [EC], F32, kind="ExternalInput")
    bkd = nc.dram_tensor("bk", [EC], F32, kind="ExternalInput")
    bvd = nc.dram_tensor("bv", [EC], BF16, kind="ExternalInput")
    cst = nc.dram_tensor("cst", [128, CST_W], BF16, kind="ExternalInput")
    out = nc.dram_tensor("out", [S, E], BF16, kind="ExternalOutput")

    with tile.TileContext(nc) as tc:
        with ExitStack() as stack:
            cpool = stack.enter_context(tc.tile_pool(name="const", bufs=1))
            wpool = stack.enter_context(tc.tile_pool(name="w", bufs=1))
            xpool = stack.enter_context(tc.tile_pool(name="xt", bufs=NEC))
            qkpool = stack.enter_context(tc.tile_pool(name="qkt", bufs=4))
            vpool = stack.enter_context(tc.tile_pool(name="vsb", bufs=NKC))
            apool = stack.enter_context(tc.tile_pool(name="asb", bufs=2))
            ppool = stack.enter_context(tc.tile_pool(name="psb", bufs=18))
            rspool = stack.enter_context(tc.tile_pool(name="rs", bufs=2))
            bcpool = stack.enter_context(tc.tile_pool(name="bc", bufs=2))
            opool = stack.enter_context(tc.tile_pool(name="osb", bufs=4))
            pj_ps = stack.enter_context(tc.tile_pool(name="pj_ps", bufs=2, space="PSUM"))
            qk_ps = stack.enter_context(tc.tile_pool(name="qk_ps", bufs=2, space="PSUM"))
            at_ps = stack.enter_context(tc.tile_pool(name="at_ps", bufs=2, space="PSUM"))

            # ---- weight + input DMAs ----
            # few large queue entries (the DGE burns ~600ns per entry), ordered
            # so the first attention section's inputs land first: wq, wk, then
            # the first column-half of xT, then constants/wv/wo, then the rest.
            w_sb = {}
            w_big = {}
            for name, dram in (("q", wq), ("k", wk), ("v", wv)):
                big = wpool.tile([128, NEC * EC], BF16, tag=f"w{name}")
                w_big[name] = (big, dram)
                w_sb[name] = [big[:, e * EC:(e + 1) * EC] for e in range(NEC)]

            def w_dma(name):
                big, dram = w_big[name]
                nc.sync.dma_start(
                    big[:].rearrange("p (c n) -> p c n", c=NEC),
                    dram.ap().rearrange("(c p) n -> p c n", p=128),
                )

            bq_sb = cpool.tile([128, 2], F32, tag="bq")
            nc.sync.dma_start(bq_sb[:], bqd.ap().rearrange("(b p) -> p b", p=128))
            # preload the exp table set immediately (it costs ~2.7us once)
            dummy0 = cpool.tile([1, 1], F32, tag="dummy0")
            nc.scalar.activation(dummy0[:], bq_sb[0:1, 0:1], EXP)
            # HAM warmup: ~128 tiny matmuls keep the PE "busy" through the
            # DMA-bound startup so the clock gate opens before real matmuls.
            # Uses a qk-pool psum tile (idle until the first scores at ~20us)
            # so the projection units' pj pool is never blocked behind it.
            warm_ps = qk_ps.tile([128, 1024], F32, tag="qk")
            for _ in range(128):
                nc.tensor.matmul(
                    warm_ps[0:2, 0:2], bq_sb[:, 0:2], bq_sb[:, 0:2],
                    start=True, stop=True,
                )

            w_dma("q")
            w_dma("k")
            xt_sb = [
                xpool.tile([128, S], BF16, tag="xt", name=f"xt{e}")
                for e in range(NEC)
            ]
            for e in range(NEC):
                nc.sync.dma_start(xt_sb[e][:, 0:1024], xT[e * 128:(e + 1) * 128, 0:1024])

            cst_sb = cpool.tile([128, CST_W], BF16, tag="cst")
            nc.sync.dma_start(cst_sb[:], cst[:])
            mask_sb = cst_sb[:, 0:128]
            static_blk = cst_sb[:, 128:193]
            bk_sb = cpool.tile([128, 2], F32, tag="bk")
            nc.sync.dma_start(bk_sb[:], bkd.ap().rearrange("(b p) -> p b", p=128))
            bv_sb = cpool.tile([1, EC], BF16, tag="bv")
            nc.sync.dma_start(bv_sb[:], bvd.ap().rearrange("(o n) -> o n", o=1))

            w_dma("v")
            wo_big = cpool.tile([128, 2 * E], BF16, tag="wo")
            nc.sync.dma_start(
                wo_big[:].rearrange("p (c n) -> p c n", c=2),
                wo.ap().rearrange("(c p) n -> p c n", p=128),
            )
            wo_sb = [wo_big[:, j * E:(j + 1) * E] for j in range(2)]
            for e in range(NEC):
                nc.sync.dma_start(
                    xt_sb[e][:, 1024:2048], xT[e * 128:(e + 1) * 128, 1024:2048]
                )

            # bv broadcast [128, EC] = ones[1,128].T @ bv[1,EC]
            # (mask row 0 is all-ones and doubles as the ones lhsT row)
            bvb_ps = pj_ps.tile([128, 512], F32, tag="pj")
            nc.tensor.matmul(
                bvb_ps[:, 0:EC], cst_sb[0:1, 0:128], bv_sb[:], start=True, stop=True
            )
            bvb_sb = cpool.tile([128, EC], F32, tag="bvb")
            nc.vector.tensor_copy(bvb_sb[:], bvb_ps[:, 0:EC])



            qt_sb = [qkpool.tile([128, S], BF16, tag="qkt", name=f"qt{i}") for i in range(2)]
            kt_sb = [qkpool.tile([128, S], BF16, tag="qkt", name=f"kt{i}") for i in range(2)]
            a_sb = [apool.tile([128, S], BF16, tag="asb", name=f"a{i}") for i in range(2)]

            def qk_unit(name, dst, pb, ti, bias_sb):
                """One q/k projection unit: dst[pb][:, ti*512:+512] (d on partitions)."""
                ps = pj_ps.tile([128, 512], F32, tag="pj")
                for e in range(NEC):
                    nc.tensor.matmul(
                        ps[:],
                        w_sb[name][e][:, pb * 128:(pb + 1) * 128],
                        xt_sb[e][:, ti * 512:(ti + 1) * 512],
                        start=(e == 0),
                        stop=(e == NEC - 1),
                    )
                nc.vector.tensor_scalar_add(
                    dst[pb][:, ti * 512:(ti + 1) * 512], ps[:], bias_sb[:, pb:pb + 1]
                )

            v_sb = []

            def v_unit(m):
                """v projection for k-chunk m (k on partitions, PV-ready layout)."""
                vt = vpool.tile([128, V_W], BF16, tag="vsb")
                for cs in V_STATIC:
                    nc.vector.tensor_copy(vt[:, cs:cs + 65], static_blk)
                vps = pj_ps.tile([128, 512], F32, tag="pj")
                for e in range(NEC):
                    nc.tensor.matmul(
                        vps[:, 0:EC],
                        xt_sb[e][:, m * 128:(m + 1) * 128],
                        w_sb["v"][e][:],
                        start=(e == 0),
                        stop=(e == NEC - 1),
                    )
                for h in range(HPC):
                    d0 = V_DATA[h]
                    nc.vector.tensor_add(
                        vt[:, d0:d0 + 64],
                        vps[:, h * 64:(h + 1) * 64],
                        bvb_sb[:, h * 64:(h + 1) * 64],
                    )
                v_sb.append(vt)

            def oproj_unit(m):
                """out[m*128:+128, :] = sum_j a_sb[j].T @ wo_sb[j], staged into
                one [128, 1024] row-contiguous DMA (2KB rows, full bandwidth)."""
                osb = opool.tile([128, E], BF16, tag="osb")
                for nh in range(2):
                    ops = pj_ps.tile([128, 512], F32, tag="pj")
                    for j in range(2):
                        nc.tensor.matmul(
                            ops[:],
                            a_sb[j][:, m * 128:(m + 1) * 128],
                            wo_sb[j][:, nh * 512:(nh + 1) * 512],
                            start=(j == 0),
                            stop=(j == 1),
                        )
                    nc.vector.tensor_copy(osb[:, nh * 512:(nh + 1) * 512], ops[:])
                nc.sync.dma_start(out[m * 128:(m + 1) * 128, :], osb[:])

            def attn_section(p, ti, mid=None):
                nchunks = 4 * (ti + 1)
                lhs = (V_LHS[2 * p], V_LHS[2 * p + 1])
                ape = at_ps.tile([128, 512], F32, tag="at", name="ape")
                apo = at_ps.tile([128, 512], F32, tag="at", name="apo")
                tgt = (ape[0:65, :], apo[:, :])

                # phase 1: all scores + exp + mask, so the exp stream never
                # queues behind PV matmuls that wait on the previous section's
                # normalize chain (ape/apo pool release)
                psbs = []
                for ci in range(nchunks):
                    d0 = ci * 128 - ti * 512
                    q0 = max(d0, 0)
                    qkp = qk_ps.tile([128, 1024], F32, tag="qk")
                    psb = ppool.tile([128, 1024], BF16, tag="psb")
                    psbs.append((psb, q0))
                    for hh in range(2):
                        nc.tensor.matmul(
                            qkp[:, 512 * hh + q0:512 * hh + 512],
                            kt_sb[p][64 * hh:64 * hh + 64, ci * 128:(ci + 1) * 128],
                            qt_sb[p][64 * hh:64 * hh + 64, ti * 512 + q0:(ti + 1) * 512],
                            start=True, stop=True,
                        )
                    if q0 == 0:
                        nc.scalar.activation(psb[:], qkp[:], EXP, scale=0.125)
                    else:
                        nc.scalar.activation(
                            psb[:, q0:512], qkp[:, q0:512], EXP, scale=0.125)
                        nc.scalar.activation(
                            psb[:, 512 + q0:1024], qkp[:, 512 + q0:1024], EXP,
                            scale=0.125)
                    if d0 >= 0:
                        for hh in range(2):
                            nc.vector.tensor_mul(
                                psb[:, 512 * hh + q0:512 * hh + q0 + 128],
                                psb[:, 512 * hh + q0:512 * hh + q0 + 128],
                                mask_sb,
                            )
                # filler emission point: lower priority than the exp stream
                # above, higher than the PV drains below
                if mid is not None:
                    mid()
                # phase 2: PV accumulation trails the exp stream
                for ci in range(nchunks):
                    psb, q0 = psbs[ci]
                    for hh in range(2):
                        lh = lhs[hh]
                        nc.tensor.matmul(
                            tgt[hh][:, q0:512],
                            v_sb[ci][:, lh[0]:lh[1]],
                            psb[:, 512 * hh + q0:512 * hh + 512],
                            start=(ci == 0),
                            stop=(ci == nchunks - 1),
                        )

                # softmax normalization: stage denominators to SBUF, fast
                # recip, then broadcast via two K=1 col-tiled PE matmuls
                # (ones-row x recip-row -> partitions 0:64 / 64:128 of one
                # psum tile) + one scalar-engine copy.  No DMA: the replicate
                # DMA costs ~5us of descriptor processing per section.
                # (reciprocal_approx_fast misbehaves on a PSUM source on HW.)
                # den rows staged to SBUF as bf16; ones rows come from the
                # mask constant (row 0 all-ones; row 64 ones on cols 64:128)
                ssb = rspool.tile([128, 512], BF16, tag="ssb", name="ssb")
                nc.vector.tensor_copy(ssb[64:65, :], ape[64:65, :])
                nc.vector.tensor_copy(ssb[0:1, :], apo[0:1, :])
                bcp = pj_ps.tile([128, 512], F32, tag="pj")
                nc.tensor.matmul(
                    bcp[0:64, :], cst_sb[64:65, 64:128], ssb[64:65, :],
                    start=True, stop=True,
                )
                nc.tensor.matmul(
                    bcp[64:128, :], cst_sb[0:1, 0:64], ssb[0:1, :],
                    start=True, stop=True,
                )
                bcs = bcpool.tile([128, 512], F32, tag="bc", name="bcs")
                nc.scalar.copy(bcs[:], bcp[:])
                rsf = bcpool.tile([128, 512], F32, tag="rsf", name="rsf")
                nc.vector.reciprocal_approx_fast(out=rsf[:], in_=bcs[:])
                tcols = slice(ti * 512, (ti + 1) * 512)
                nc.vector.tensor_mul(a_sb[p][0:64, tcols], ape[0:64, :], rsf[0:64, :])
                nc.vector.tensor_mul(a_sb[p][64:128, tcols], apo[64:128, :], rsf[64:128, :])

            # ---- emission order = scheduler priority ----
            # each section's phase-1 (scores+exp) is preceded only by the q/k
            # units it needs; v-projections and out-projections are emitted at
            # the section's mid-point so they fill PE slack during the exp
            # stream without ever delaying it.
            def mk_mid(vs, qks, ops):
                def mid():
                    for name, pb, ti in qks:
                        qk_unit(name, qt_sb if name == "q" else kt_sb, pb, ti,
                                bq_sb if name == "q" else bk_sb)
                    for m in vs:
                        v_unit(m)
                    for m in ops:
                        oproj_unit(m)
                return mid

            qk_unit("q", qt_sb, 0, 0, bq_sb)
            qk_unit("k", kt_sb, 0, 0, bk_sb)
            attn_section(0, 0, mk_mid([0, 1, 2, 3], [("q", 1, 0), ("k", 1, 0)], []))
            attn_section(1, 0, mk_mid([], [("q", 0, 1), ("k", 0, 1)], []))
            attn_section(0, 1, mk_mid([4, 5, 6, 7], [("q", 1, 1), ("k", 1, 1)], []))
            attn_section(1, 1, mk_mid([], [("q", 0, 2), ("k", 0, 2)], [0, 1, 2, 3]))
            attn_section(0, 2, mk_mid([8, 9, 10, 11], [("q", 1, 2), ("k", 1, 2)], []))
            attn_section(1, 2, mk_mid([], [("q", 0, 3), ("k", 0, 3)], [4, 5, 6, 7]))
            attn_section(0, 3, mk_mid([12, 13, 14, 15], [("q", 1, 3), ("k", 1, 3)], []))
            attn_section(1, 3, mk_mid([], [], [8, 9, 10, 11]))
            for m in range(12, 16):
                oproj_unit(m)

    nc.compile()
    return nc


_NC = None


def _get_nc():
    global _NC
    if _NC is None:
        _NC = _build_nc()
    return _NC


def _constants():
    kk = np.arange(128)[:, None]
    jj = np.arange(128)[None, :]
    cst = np.zeros((128, CST_W), dtype=np.float32)
    cst[:, 0:128] = (jj >= kk).astype(np.float32)
    cst[:, 128] = 1.0
    cst[:, 129] = 1.0
    return cst.astype(ml_dtypes.bfloat16)


def _in_maps(inputs, Wq, bq, Wk, bk, Wv, bv, Wo, bo):
    bf = ml_dtypes.bfloat16
    inputs = np.asarray(inputs, dtype=np.float32)
    Wq = np.asarray(Wq, dtype=np.float32)
    Wk = np.asarray(Wk, dtype=np.float32)
    Wv = np.asarray(Wv, dtype=np.float32)
    Wo = np.asarray(Wo, dtype=np.float32)
    bq = np.asarray(bq, dtype=np.float32)
    bk = np.asarray(bk, dtype=np.float32)
    bv = np.asarray(bv, dtype=np.float32)

    cst = _constants()
    xTb = [np.ascontiguousarray(inputs[b].T).astype(bf) for b in range(B)]
    maps = []
    for c in range(8):
        b, g = divmod(c, 4)
        sl = slice(g * EC, (g + 1) * EC)
        maps.append({
            "xT": xTb[b],
            "wq": np.ascontiguousarray(Wq[:, sl]).astype(bf),
            "bq": np.ascontiguousarray(bq[sl]),
            "wk": np.ascontiguousarray(Wk[:, sl]).astype(bf),
            "bk": np.ascontiguousarray(bk[sl]),
            "wv": np.ascontiguousarray(Wv[:, sl]).astype(bf),
            "bv": np.ascontiguousarray(bv[sl]).astype(bf),
            "wo": np.ascontiguousarray(Wo[sl, :]).astype(bf),
            "cst": cst,
        })
    return maps


def _assemble(results, bo):
    bo = np.asarray(bo, dtype=np.float32)
    outs = [np.asarray(r["out"]).astype(np.float32) for r in results]
    full = np.empty((B, S, E), dtype=np.float32)
    for b in range(B):
        full[b] = outs[4 * b] + outs[4 * b + 1] + outs[4 * b + 2] + outs[4 * b + 3]
        full[b] += bo
    return full


def kernel(inputs, Wq, bq, Wk, bk, Wv, bv, Wo, bo):
    nc = _get_nc()
    maps = _in_maps(inputs, Wq, bq, Wk, bk, Wv, bv, Wo, bo)
    res = run_bass_kernel_spmd(nc, maps, list(range(8)))
    return _assemble(res.results, bo)
